# revision 24
# baseline (speedup 1.0000x reference)
"""Trainium2 Bass kernel for the STU (spectral transform unit) dense-transformer block.

Algorithm (validated against the jax reference in fp64 numpy):
  The FFT causal conv is rewritten as a block-Toeplitz matmul. For each of the
  K=16 filters and each sign branch (the alternating-sign branch folds into the
  filter taps: T^-[s,s'] = phi[s-s'] * (-1)^(s-s')), the causal conv is
    U_br = T_br @ u,  T_br block-Toeplitz with 16 distinct 128x128 blocks.
  sigma^(1/4) folds into the taps. The (k,i)->d projection contracts U with
  M_phi_{plus,minus}; the KU=3 autoregressive taps are shifted-u projections
  with M_u. MLP is a standard gated MLP.

Sharding (8 cores, no cross-core communication, host-side reduce between two
uniform SPMD programs):
  Phase 1: filter-branch-parallel. Core c computes conv + projection for its 4
           of the 32 (k, sign) branches over the full (B, SL): partial spectral.
  Host:    x1 = x + sum_c partial_c
  Phase 2: row-parallel. Core c owns 512 of the 4096 (b, s) rows: adds the AR
           term and computes the gated MLP + residual for its rows.

Precision: the conv runs in bf16 (its output feeds values of magnitude ~0.05,
so bf16 noise is negligible); every O(1)-magnitude contraction (projection,
AR, fc1, fc2) runs in float32r — fp32 storage at full PE rate for moving
dims >= 256, measured ~15x more accurate than bf16.
"""

import numpy as np
import ml_dtypes

import concourse.bacc as bacc
import concourse.tile as tile
from concourse import mybir
from concourse.bass_utils import run_bass_kernel_spmd  # noqa: F401 (debug path)
from concourse.masks import make_identity


class _SpmdRunner:
    """Cached-jit SPMD executor: trace/compile once, then repeat calls only
    pay input upload + execution (mirrors bass2jax.run_bass_via_pjrt).

    ``shared`` names inputs that are identical on every core: they are fed
    replicated (host uploads one copy) instead of 8x-concatenated."""

    def __init__(self, nc, shared=(), volatile=()):
        import jax
        import concourse.mybir as _mb
        from concourse.bass2jax import (
            install_neuronx_cc_hook, _bass_exec_p, partition_id_tensor,
        )
        from jax.experimental.shard_map import shard_map
        from jax.sharding import Mesh, PartitionSpec

        install_neuronx_cc_hook()
        self.nc = nc
        assert nc.dbg_addr is None
        pid_name = (nc.partition_id_tensor.name
                    if nc.partition_id_tensor is not None else None)
        in_names, out_names, out_avals = [], [], []
        for alloc in nc.m.functions[0].allocations:
            if not isinstance(alloc, mybir.MemoryLocationSet):
                continue
            name = alloc.memorylocations[0].name
            if alloc.kind == "ExternalInput":
                if name != pid_name:
                    in_names.append(name)
            elif alloc.kind == "ExternalOutput":
                out_names.append(name)
                out_avals.append(jax.core.ShapedArray(
                    tuple(alloc.tensor_shape), mybir.dt.np(alloc.dtype)))
        self.in_names, self.out_names, self.out_avals = in_names, out_names, out_avals
        self.shared = frozenset(shared)
        self.volatile = frozenset(volatile)
        self._dev_cache = {}
        n_params = len(in_names)
        all_names = tuple(in_names + out_names)
        if pid_name is not None:
            all_names = all_names + (pid_name,)

        def _body(*args):
            args = list(args)
            if pid_name is not None:
                args.append(partition_id_tensor())
            return tuple(_bass_exec_p.bind(
                *args,
                out_avals=tuple(out_avals),
                in_names=all_names,
                out_names=tuple(out_names),
                lowering_input_output_aliases=(),
                sim_require_finite=True,
                sim_require_nnan=True,
                nc=nc,
            ))

        import jax.numpy as jnp
        from jax.sharding import NamedSharding
        devices = jax.devices()[:NCORES]
        mesh = Mesh(np.asarray(devices), ("core",))
        rep = PartitionSpec()
        core = PartitionSpec("core")
        in_specs = tuple(
            rep if nm in self.shared else core for nm in in_names
        ) + (core,) * len(out_names)
        out_specs = (core,) * len(out_names)
        donate = tuple(range(n_params, n_params + len(out_names)))
        self._fn = jax.jit(
            shard_map(_body, mesh=mesh, in_specs=in_specs, out_specs=out_specs,
                      check_rep=False),
            donate_argnums=donate, keep_unused=True,
        )
        self._zeros_fn = jax.jit(
            lambda: tuple(
                jnp.zeros((NCORES * a.shape[0], *a.shape[1:]), a.dtype)
                for a in out_avals
            ),
            out_shardings=tuple(
                NamedSharding(mesh, core) for _ in out_avals
            ),
        )
        self._shardings = {
            nm: NamedSharding(mesh, rep if nm in self.shared else core)
            for nm in in_names
        }

    def prep(self, in_maps):
        import hashlib
        import jax
        ins = []
        for nm in self.in_names:
            if nm in self.shared:
                arr = np.ascontiguousarray(in_maps[0][nm])
            else:
                arr = np.concatenate(
                    [np.asarray(in_maps[c][nm]) for c in range(NCORES)], axis=0)
            if nm in self.volatile:
                ins.append(arr)
                continue
            key = (nm, hashlib.md5(arr.tobytes()).hexdigest())
            dev = self._dev_cache.get(key)
            if dev is None:
                self._dev_cache.clear() if len(self._dev_cache) > 32 else None
                dev = jax.device_put(arr, self._shardings[nm])
                self._dev_cache[key] = dev
            ins.append(dev)
        return ins

    def run_prepped(self, ins):
        return self._fn(*ins, *self._zeros_fn())

    def __call__(self, in_maps):
        out_arrs = self.run_prepped(self.prep(in_maps))
        return [
            {nm: np.asarray(out_arrs[i]).reshape(NCORES, *self.out_avals[i].shape)[c]
             for i, nm in enumerate(self.out_names)}
            for c in range(NCORES)
        ]

BF16 = ml_dtypes.bfloat16
FP8NP = ml_dtypes.float8_e4m3
TAP_SCALE = 1024.0
UT_SCALE = 32.0      # psum (TAP_SCALE*U) -> fp8 ut tiles scale factor: 32/1024
W_SCALE = 16.0       # projection weights scaled by 16 for fp8 range
SP_SCALE = UT_SCALE * W_SCALE  # spectral psum carries 32*16 = 512x
F32 = mybir.dt.float32
F32R = mybir.dt.float32r
BF = mybir.dt.bfloat16
FP8 = mybir.dt.float8e4

B, SL, D, K, KU = 2, 2048, 768, 16, 3
NFFT, EPS, P, H = 4096, 1e-5, 128, 3072
NB = SL // P            # 16 seq blocks
DC = D // P             # 6 d-chunks
NBR = 2 * K             # 32 conv branches
NCORES = 8
BPC = NBR // NCORES     # 4 branches per core
RPC = (B * SL) // NCORES  # 512 rows per core
MB = RPC // P           # 4 row blocks per core in phase 2
JC = H // P             # 24 hidden chunks
F1 = 512                # free-dim split of D=768 into 512+256

_cache: dict = {}


def _mm_r(nc, out, lhsT, rhs, start, stop):
    nc.tensor.matmul(out, lhsT=lhsT, rhs=rhs, start=start, stop=stop)


def _build_phase1(skip_conv=False, skip_proj=False, skip_norm=False):
    nc = bacc.Bacc("TRN2", target_bir_lowering=False, debug=False, num_devices=NCORES)
    x = nc.dram_tensor("x", (B, SL, D), F32, kind="ExternalInput").ap()
    tw = nc.dram_tensor("tw", (NB, P, 2, BPC * P), FP8, kind="ExternalInput").ap()
    wt = nc.dram_tensor("wt", (BPC, DC // 2, P, 2, D), FP8, kind="ExternalInput").ap()
    rn1 = nc.dram_tensor("rn1", (1, D), F32, kind="ExternalInput").ap()
    sp = nc.dram_tensor("sp", (B, SL, D), F32, kind="ExternalOutput").ap()

    with tile.TileContext(nc) as tc:
        with (
            tc.tile_pool(name="const", bufs=1) as const_pool,
            tc.tile_pool(name="ubuf", bufs=1) as ubuf_pool,
            tc.tile_pool(name="work", bufs=3) as work,
            tc.tile_pool(name="drain", bufs=2) as drain_pool,
            tc.tile_pool(name="psum_u", bufs=3, space="PSUM") as psum_u_pool,
            tc.tile_pool(name="psum_sp", bufs=2, space="PSUM") as psum_sp_pool,
        ):
            tw_sb = const_pool.tile([P, NB, 2, BPC * P], FP8)
            nc.sync.dma_start(tw_sb, tw.rearrange("d p k f -> p d k f"))
            wt_sb = const_pool.tile([P, BPC, DC // 2, 2, D], FP8)
            nc.sync.dma_start(wt_sb, wt.rearrange("b c p k f -> p b c k f"))
            rn1_bc = const_pool.tile([P, D], F32)
            nc.sync.dma_start(rn1_bc, rn1.to_broadcast((P, D)))
            eps_sb = const_pool.tile([P, 1], F32)
            nc.vector.memset(eps_sb, float(EPS))

            # u = rmsnorm(x) * rn1_w, cast to bf16, for all (b, J)
            u_all = []
            for b in range(B):
                u_all.append(ubuf_pool.tile([P, NB, D], FP8, name=f"u{b}"))
            for b in range(B):
                if skip_norm:
                    break
                for J in range(NB):
                    xt = work.tile([P, D], F32, name="xt")
                    nc.sync.dma_start(xt, x[b, J * P:(J + 1) * P, :])
                    sq = work.tile([P, D], F32, name="sq")
                    ms = work.tile([P, 1], F32, name="ms")
                    nc.scalar.activation(
                        sq, xt, mybir.ActivationFunctionType.Square, accum_out=ms
                    )
                    nc.scalar.activation(
                        ms, ms, mybir.ActivationFunctionType.Sqrt,
                        bias=eps_sb, scale=1.0 / D,
                    )
                    nc.vector.reciprocal(ms, ms)
                    nc.vector.tensor_scalar_mul(xt, xt, ms)
                    nc.vector.tensor_tensor(
                        u_all[b][:, J, :], xt, rn1_bc, mybir.AluOpType.mult
                    )

            # conv (block-Toeplitz, bf16) + projection (f32r) per (b, I)
            for b in range(B):
                for I in range(NB):
                    ut_sb = drain_pool.tile([P, DC, BPC * P], FP8, name="ut")
                    if skip_conv:
                        nc.vector.memset(ut_sb, 0.0)
                    for c in range(DC if not skip_conv else 0):
                        ps = psum_u_pool.tile([P, BPC * P], F32, name="psu")
                        npair = I // 2 + 1
                        for Jp in range(npair):
                            nc.tensor.matmul(
                                ps,
                                lhsT=u_all[b][:, 2 * Jp:2 * Jp + 2, c * P:(c + 1) * P],
                                rhs=tw_sb[:, I - 2 * Jp, :, :],
                                start=(Jp == 0),
                                stop=(Jp == npair - 1),
                                perf_mode=mybir.MatmulPerfMode.DoubleRow,
                            )
                        nc.scalar.activation(
                            ut_sb[:, c, :], ps,
                            mybir.ActivationFunctionType.Copy,
                            scale=float(UT_SCALE / TAP_SCALE),
                        )
                    psp = psum_sp_pool.tile([P, D], F32, name="psp")
                    n_mm = BPC * (DC // 2)
                    i_mm = 0
                    for br in range(BPC if not skip_proj else 0):
                        for cp in range(DC // 2):
                            st = i_mm == 0
                            fin = i_mm == n_mm - 1
                            lh = ut_sb[:, 2 * cp:2 * cp + 2, br * P:(br + 1) * P]
                            nc.tensor.matmul(
                                psp[:, 0:F1], lhsT=lh,
                                rhs=wt_sb[:, br, cp, :, 0:F1],
                                start=st, stop=fin,
                                perf_mode=mybir.MatmulPerfMode.DoubleRow,
                            )
                            nc.tensor.matmul(
                                psp[:, F1:D], lhsT=lh,
                                rhs=wt_sb[:, br, cp, :, F1:D],
                                start=st, stop=fin,
                                perf_mode=mybir.MatmulPerfMode.DoubleRow,
                            )
                            i_mm += 1
                    sp_t = work.tile([P, D], F32, name="spt")
                    if skip_proj:
                        nc.vector.memset(psp, 0.0)
                    nc.scalar.activation(
                        sp_t, psp, mybir.ActivationFunctionType.Copy,
                        scale=float(1.0 / SP_SCALE),
                    )
                    nc.sync.dma_start(sp[b, I * P:(I + 1) * P, :], sp_t)
    nc.compile()
    return nc


def _build_phase2():
    nc = bacc.Bacc("TRN2", target_bir_lowering=False, debug=False, num_devices=NCORES)
    xr = nc.dram_tensor("xr", (RPC + 2, D), F32, kind="ExternalInput").ap()
    x1r = nc.dram_tensor("x1r", (RPC, D), F32, kind="ExternalInput").ap()
    mut = nc.dram_tensor("mut", (KU, DC, P, D), F32R, kind="ExternalInput").ap()
    fc1 = nc.dram_tensor("fc1", (D, 2 * H), F32R, kind="ExternalInput").ap()
    fc2 = nc.dram_tensor("fc2", (H, D), F32R, kind="ExternalInput").ap()
    rn1 = nc.dram_tensor("rn1", (1, D), F32, kind="ExternalInput").ap()
    rn2 = nc.dram_tensor("rn2", (1, D), F32, kind="ExternalInput").ap()
    o = nc.dram_tensor("o", (RPC, D), F32, kind="ExternalOutput").ap()

    fc1_r = fc1.rearrange("(c p) j -> p c j", p=P)
    fc2_r = fc2.rearrange("(c p) d -> p c d", p=P)

    with tile.TileContext(nc) as tc:
        with (
            tc.tile_pool(name="const", bufs=1) as const_pool,
            tc.tile_pool(name="persist", bufs=1) as persist,
            tc.tile_pool(name="work", bufs=2) as work,
            tc.tile_pool(name="wstream", bufs=2) as wstream,
            tc.tile_pool(name="psum_t", bufs=2, space="PSUM") as psum_t_pool,
            tc.tile_pool(name="psum_big", bufs=3, space="PSUM") as psum_big_pool,
        ):
            mut_sb = const_pool.tile([P, KU, DC, D], F32R)
            nc.sync.dma_start(mut_sb, mut.rearrange("t c p d -> p t c d"))
            rn1_bc = const_pool.tile([P, D], F32)
            nc.sync.dma_start(rn1_bc, rn1.to_broadcast((P, D)))
            rn2_bc = const_pool.tile([P, D], F32)
            nc.sync.dma_start(rn2_bc, rn2.to_broadcast((P, D)))
            ident = const_pool.tile([P, P], F32)
            make_identity(nc, ident)
            eps_sb = const_pool.tile([P, 1], F32)
            nc.vector.memset(eps_sb, float(EPS))

            u_pre = persist.tile([2, D], F32)
            ut_ext = persist.tile([P, DC, MB, P + 2], F32R)
            x1p = persist.tile([P, MB, D], F32)
            yt = persist.tile([P, DC, MB * P], F32R)
            gt = persist.tile([P, JC, MB * P], F32R)

            def rmsnorm_to(dst, src_f32, rows, w_bc):
                sq = work.tile([P, D], F32, name="sq")
                ms = work.tile([P, 1], F32, name="ms")
                nc.scalar.activation(
                    sq[:rows], src_f32[:rows],
                    mybir.ActivationFunctionType.Square, accum_out=ms[:rows],
                )
                nc.scalar.activation(
                    ms[:rows], ms[:rows], mybir.ActivationFunctionType.Sqrt,
                    bias=eps_sb[:rows], scale=1.0 / D,
                )
                nc.vector.reciprocal(ms[:rows], ms[:rows])
                tmp = sq  # sq is dead after the accumulated Square
                nc.vector.tensor_scalar_mul(tmp[:rows], src_f32[:rows], ms[:rows])
                nc.vector.tensor_tensor(
                    dst, tmp[:rows], w_bc[:rows], mybir.AluOpType.mult
                )

            # u for the 2-row prefix, then u^T per owned block via PE transpose
            xp = work.tile([P, D], F32, name="xt")[:2]
            nc.sync.dma_start(xp, xr[0:2, :])
            rmsnorm_to(u_pre, xp, 2, rn1_bc)
            for c in range(DC):
                pst2 = psum_t_pool.tile([P, P], F32, name="pst")
                nc.tensor.transpose(
                    pst2[:, 0:2], u_pre[:, c * P:(c + 1) * P], ident[0:2, 0:2]
                )
                nc.vector.tensor_copy(ut_ext[:, c, 0, 0:2], pst2[:, 0:2])
            for m in range(MB):
                xt = work.tile([P, D], F32, name="xt")
                nc.sync.dma_start(xt, xr[2 + m * P: 2 + (m + 1) * P, :])
                uo = work.tile([P, D], F32, name="uo")
                rmsnorm_to(uo, xt, P, rn1_bc)
                for c in range(DC):
                    pst = psum_t_pool.tile([P, P], F32, name="pst")
                    nc.tensor.transpose(pst, uo[:, c * P:(c + 1) * P], ident)
                    nc.vector.tensor_copy(ut_ext[:, c, m, 2:P + 2], pst)
            for m in range(1, MB):
                for c in range(DC):
                    nc.vector.tensor_copy(
                        ut_ext[:, c, m, 0:2], ut_ext[:, c, m - 1, P:P + 2]
                    )

            # AR term + x1 rows
            for m in range(MB):
                psa = psum_big_pool.tile([P, D], F32, name="pbig")
                i_mm = 0
                n_mm = KU * DC
                for t in range(KU):
                    for c in range(DC):
                        st = i_mm == 0
                        fin = i_mm == n_mm - 1
                        _mm_r(nc, psa[:, 0:F1],
                              ut_ext[:, c, m, 2 - t:P + 2 - t],
                              mut_sb[:, t, c, 0:F1], st, fin)
                        _mm_r(nc, psa[:, F1:D],
                              ut_ext[:, c, m, 2 - t:P + 2 - t],
                              mut_sb[:, t, c, F1:D], st, fin)
                        i_mm += 1
                x1t = work.tile([P, D], F32, name="x1t")
                nc.sync.dma_start(x1t, x1r[m * P:(m + 1) * P, :])
                nc.vector.tensor_tensor(
                    x1p[:, m, :], x1t, psa, mybir.AluOpType.add
                )

            # y = rmsnorm2(x1) and y^T
            for m in range(MB):
                yf = work.tile([P, D], F32, name="uo")
                rmsnorm_to(yf, x1p[:, m, :], P, rn2_bc)
                for c in range(DC):
                    pst = psum_t_pool.tile([P, P], F32, name="pst")
                    nc.tensor.transpose(pst, yf[:, c * P:(c + 1) * P], ident)
                    nc.vector.tensor_copy(yt[:, c, m * P:(m + 1) * P], pst)

            # fc1 + silu gate
            for jc in range(JC):
                fw = wstream.tile([P, DC, 2, P], F32R, name="fw")
                nc.sync.dma_start(fw[:, :, 0, :], fc1_r[:, :, jc * P:(jc + 1) * P])
                nc.sync.dma_start(
                    fw[:, :, 1, :], fc1_r[:, :, (JC + jc) * P:(JC + jc + 1) * P]
                )
                ph1 = psum_big_pool.tile([P, D], F32, name="pbig")[:, 0:F1]
                ph2 = psum_big_pool.tile([P, D], F32, name="pbig")[:, 0:F1]
                for c in range(DC):
                    _mm_r(nc, ph1, fw[:, c, 0, :], yt[:, c, :],
                          c == 0, c == DC - 1)
                    _mm_r(nc, ph2, fw[:, c, 1, :], yt[:, c, :],
                          c == 0, c == DC - 1)
                sact = work.tile([P, F1], F32, name="sact")
                nc.scalar.activation(sact, ph2, mybir.ActivationFunctionType.Silu)
                nc.vector.tensor_tensor(
                    gt[:, jc, :], ph1, sact, mybir.AluOpType.mult
                )

            # fc2 + residual: m in pairs so each fc2 chunk is loaded once per
            # two row-blocks (fc2 streamed twice total instead of 4x)
            for mp in range(MB // 2):
                po2 = [
                    psum_big_pool.tile([P, D], F32, name="pbig")
                    for _ in range(2)
                ]
                for jc in range(JC):
                    f2w = wstream.tile([P, D], F32R, name="f2w")
                    nc.sync.dma_start(f2w, fc2_r[:, jc, :])
                    st = jc == 0
                    fin = jc == JC - 1
                    for mi in range(2):
                        m = 2 * mp + mi
                        _mm_r(nc, po2[mi][:, 0:F1],
                              gt[:, jc, m * P:(m + 1) * P], f2w[:, 0:F1], st, fin)
                        _mm_r(nc, po2[mi][:, F1:D],
                              gt[:, jc, m * P:(m + 1) * P], f2w[:, F1:D], st, fin)
                for mi in range(2):
                    m = 2 * mp + mi
                    ot = work.tile([P, D], F32, name="x1t")
                    nc.vector.tensor_tensor(
                        ot, x1p[:, m, :], po2[mi], mybir.AluOpType.add
                    )
                    nc.sync.dma_start(o[m * P:(m + 1) * P, :], ot)
    nc.compile()
    return nc


def _host_prep(V, sigma, M_u, M_phi_plus, M_phi_minus):
    """Per-core weight tensors: Toeplitz tap blocks + projection matrices."""
    phi = np.fft.irfft(V.astype(np.complex128), n=NFFT, axis=0)[:SL]
    s4 = sigma.astype(np.float64) ** 0.25
    alt = (-1.0) ** np.arange(SL)

    taps = np.zeros((NBR, SL))
    Wb = np.zeros((NBR, D, D), np.float32)
    for k in range(K):
        taps[2 * k] = s4[k] * phi[:, k]
        taps[2 * k + 1] = s4[k] * phi[:, k] * alt
        Wb[2 * k] = M_phi_plus[k]
        Wb[2 * k + 1] = M_phi_minus[k]

    idx = np.arange(P)
    cmr = idx[None, :] - idx[:, None]       # [r, c] = c - r
    tw_cores = []
    wt_cores = []
    for core in range(NCORES):
        brs = range(core * BPC, (core + 1) * BPC)
        # tw[d0, :, ko, :] = T-block pair (delta=d0 for ko=0, delta=d0-1 for
        # ko=1, zeros for delta<0), taps scaled by TAP_SCALE for fp8 range
        tw = np.zeros((NB, P, 2, BPC * P), np.float32)
        wt = np.zeros((BPC, DC // 2, P, 2, D), np.float32)
        for bi, br in enumerate(brs):
            tsc = taps[br] * TAP_SCALE
            for d0 in range(NB):
                for ko in range(2):
                    d = d0 - ko
                    if d < 0:
                        continue
                    ii = d * P + cmr
                    blk = np.where(ii >= 0, tsc[np.clip(ii, 0, SL - 1)], 0.0)
                    tw[d0, :, ko, bi * P:(bi + 1) * P] = blk
            for cp in range(DC // 2):
                for ko in range(2):
                    c = 2 * cp + ko
                    # wt[bi, cp, i, ko, d] = Wb[br][d, c*P + i] * W_SCALE
                    wt[bi, cp, :, ko, :] = Wb[br][:, c * P:(c + 1) * P].T * W_SCALE
        tw_cores.append(tw.astype(FP8NP))
        wt_cores.append(wt.astype(FP8NP))
    return tw_cores, wt_cores


def kernel(x, V, sigma, M_u, M_phi_plus, M_phi_minus, rn1_w, rn2_w, fc1_w, fc2_w):
    x = np.ascontiguousarray(x, np.float32)
    if "p1" not in _cache:
        _cache["p1"] = _SpmdRunner(_build_phase1(), shared=("x", "rn1"), volatile=("x",))
    if "p2" not in _cache:
        _cache["p2"] = _SpmdRunner(_build_phase2(), shared=("mut", "fc1", "fc2", "rn1", "rn2"), volatile=("xr", "x1r"))

    tw_cores, wt_cores = _host_prep(V, sigma, M_u, M_phi_plus, M_phi_minus)
    rn1 = np.ascontiguousarray(rn1_w, np.float32).reshape(1, D)
    rn2 = np.ascontiguousarray(rn2_w, np.float32).reshape(1, D)

    in_maps1 = [
        {"x": x, "tw": tw_cores[c], "wt": wt_cores[c], "rn1": rn1}
        for c in range(NCORES)
    ]
    r1 = _cache["p1"]
    sp_cat = r1.run_prepped(r1.prep(in_maps1))[0]
    if "reduce" not in _cache:
        import jax
        import jax.numpy as jnp
        from jax.sharding import NamedSharding, PartitionSpec
        mesh = r1._fn.__wrapped__ if False else None
        sh = NamedSharding(r1._shardings["x"].mesh, PartitionSpec())
        _cache["reduce"] = jax.jit(
            lambda spc, xx: xx + spc.reshape(NCORES, B, SL, D).sum(0),
            out_shardings=sh,
        )
    x1 = np.asarray(_cache["reduce"](sp_cat, r1.prep(in_maps1)[0]
                                     if False else np.asarray(x)))

    # phase 2 inputs
    mut = np.zeros((KU, DC, P, D), np.float32)
    for t in range(KU):
        for c in range(DC):
            mut[t, c] = M_u[t][:, c * P:(c + 1) * P].T
    fc1 = np.ascontiguousarray(fc1_w, np.float32)
    fc2 = np.ascontiguousarray(fc2_w, np.float32)

    x_rows = x.reshape(B * SL, D)
    x1_rows = x1.reshape(B * SL, D)
    in_maps2 = []
    for c in range(NCORES):
        r0 = c * RPC
        xr = np.zeros((RPC + 2, D), np.float32)
        xr[2:] = x_rows[r0:r0 + RPC]
        if r0 % SL != 0:
            xr[0:2] = x_rows[r0 - 2:r0]
        in_maps2.append({
            "xr": xr, "x1r": np.ascontiguousarray(x1_rows[r0:r0 + RPC]),
            "mut": mut, "fc1": fc1, "fc2": fc2, "rn1": rn1, "rn2": rn2,
        })
    res2 = _cache["p2"](in_maps2)
    out = np.concatenate(
        [res2[c]["o"] for c in range(NCORES)], axis=0
    ).reshape(B, SL, D)
    return out


# revision 28
# speedup vs baseline: 1.1924x; 1.1924x over previous
"""Trainium2 Bass kernel for the STU (spectral transform unit) dense-transformer block.

Algorithm (validated against the jax reference in fp64 numpy):
  The FFT causal conv is rewritten as a block-Toeplitz matmul. For each of the
  K=16 filters and each sign branch (the alternating-sign branch folds into the
  filter taps: T^-[s,s'] = phi[s-s'] * (-1)^(s-s')), the causal conv is
    U_br = T_br @ u,  T_br block-Toeplitz with 16 distinct 128x128 blocks.
  sigma^(1/4) folds into the taps. The (k,i)->d projection contracts U with
  M_phi_{plus,minus}; the KU=3 autoregressive taps are shifted-u projections
  with M_u. MLP is a standard gated MLP.

Sharding (8 cores, no cross-core communication, host-side reduce between two
uniform SPMD programs):
  Phase 1: filter-branch-parallel. Core c computes conv + projection for its 4
           of the 32 (k, sign) branches over the full (B, SL): partial spectral.
  Host:    x1 = x + sum_c partial_c
  Phase 2: row-parallel. Core c owns 512 of the 4096 (b, s) rows: adds the AR
           term and computes the gated MLP + residual for its rows.

Precision: the conv runs in bf16 (its output feeds values of magnitude ~0.05,
so bf16 noise is negligible); every O(1)-magnitude contraction (projection,
AR, fc1, fc2) runs in float32r — fp32 storage at full PE rate for moving
dims >= 256, measured ~15x more accurate than bf16.
"""

import numpy as np
import ml_dtypes

import concourse.bacc as bacc
import concourse.tile as tile
from concourse import mybir
from concourse.bass_utils import run_bass_kernel_spmd  # noqa: F401 (debug path)
from concourse.masks import make_identity


class _SpmdRunner:
    """Cached-jit SPMD executor: trace/compile once, then repeat calls only
    pay input upload + execution (mirrors bass2jax.run_bass_via_pjrt).

    ``shared`` names inputs that are identical on every core: they are fed
    replicated (host uploads one copy) instead of 8x-concatenated."""

    def __init__(self, nc, shared=(), volatile=()):
        import jax
        import concourse.mybir as _mb
        from concourse.bass2jax import (
            install_neuronx_cc_hook, _bass_exec_p, partition_id_tensor,
        )
        from jax.experimental.shard_map import shard_map
        from jax.sharding import Mesh, PartitionSpec

        install_neuronx_cc_hook()
        self.nc = nc
        assert nc.dbg_addr is None
        pid_name = (nc.partition_id_tensor.name
                    if nc.partition_id_tensor is not None else None)
        in_names, out_names, out_avals = [], [], []
        for alloc in nc.m.functions[0].allocations:
            if not isinstance(alloc, mybir.MemoryLocationSet):
                continue
            name = alloc.memorylocations[0].name
            if alloc.kind == "ExternalInput":
                if name != pid_name:
                    in_names.append(name)
            elif alloc.kind == "ExternalOutput":
                out_names.append(name)
                out_avals.append(jax.core.ShapedArray(
                    tuple(alloc.tensor_shape), mybir.dt.np(alloc.dtype)))
        self.in_names, self.out_names, self.out_avals = in_names, out_names, out_avals
        self.shared = frozenset(shared)
        self.volatile = frozenset(volatile)
        self._dev_cache = {}
        n_params = len(in_names)
        all_names = tuple(in_names + out_names)
        if pid_name is not None:
            all_names = all_names + (pid_name,)

        def _body(*args):
            args = list(args)
            if pid_name is not None:
                args.append(partition_id_tensor())
            return tuple(_bass_exec_p.bind(
                *args,
                out_avals=tuple(out_avals),
                in_names=all_names,
                out_names=tuple(out_names),
                lowering_input_output_aliases=(),
                sim_require_finite=True,
                sim_require_nnan=True,
                nc=nc,
            ))

        import jax.numpy as jnp
        from jax.sharding import NamedSharding
        devices = jax.devices()[:NCORES]
        mesh = Mesh(np.asarray(devices), ("core",))
        rep = PartitionSpec()
        core = PartitionSpec("core")
        in_specs = tuple(
            rep if nm in self.shared else core for nm in in_names
        ) + (core,) * len(out_names)
        out_specs = (core,) * len(out_names)
        donate = tuple(range(n_params, n_params + len(out_names)))
        self._fn = jax.jit(
            shard_map(_body, mesh=mesh, in_specs=in_specs, out_specs=out_specs,
                      check_rep=False),
            donate_argnums=donate, keep_unused=True,
        )
        self._zeros_fn = jax.jit(
            lambda: tuple(
                jnp.zeros((NCORES * a.shape[0], *a.shape[1:]), a.dtype)
                for a in out_avals
            ),
            out_shardings=tuple(
                NamedSharding(mesh, core) for _ in out_avals
            ),
        )
        self._shardings = {
            nm: NamedSharding(mesh, rep if nm in self.shared else core)
            for nm in in_names
        }

    def prep(self, in_maps):
        import hashlib
        import jax
        ins = []
        for nm in self.in_names:
            if nm in self.shared:
                arr = np.ascontiguousarray(in_maps[0][nm])
            else:
                arr = np.concatenate(
                    [np.asarray(in_maps[c][nm]) for c in range(NCORES)], axis=0)
            if nm in self.volatile:
                ins.append(arr)
                continue
            key = (nm, hashlib.md5(arr.tobytes()).hexdigest())
            dev = self._dev_cache.get(key)
            if dev is None:
                self._dev_cache.clear() if len(self._dev_cache) > 32 else None
                dev = jax.device_put(arr, self._shardings[nm])
                self._dev_cache[key] = dev
            ins.append(dev)
        return ins

    def run_prepped(self, ins):
        return self._fn(*ins, *self._zeros_fn())

    def __call__(self, in_maps):
        out_arrs = self.run_prepped(self.prep(in_maps))
        return [
            {nm: np.asarray(out_arrs[i]).reshape(NCORES, *self.out_avals[i].shape)[c]
             for i, nm in enumerate(self.out_names)}
            for c in range(NCORES)
        ]

BF16 = ml_dtypes.bfloat16
FP8NP = ml_dtypes.float8_e4m3
TAP_SCALE = 1024.0
UT_SCALE = 32.0      # psum (TAP_SCALE*U) -> fp8 ut tiles scale factor: 32/1024
W_SCALE = 16.0       # projection weights scaled by 16 for fp8 range
SP_SCALE = UT_SCALE * W_SCALE  # spectral psum carries 32*16 = 512x
F32 = mybir.dt.float32
F32R = mybir.dt.float32r
BF = mybir.dt.bfloat16
FP8 = mybir.dt.float8e4

B, SL, D, K, KU = 2, 2048, 768, 16, 3
NFFT, EPS, P, H = 4096, 1e-5, 128, 3072
NB = SL // P            # 16 seq blocks
DC = D // P             # 6 d-chunks
NBR = 2 * K             # 32 conv branches
NCORES = 8
BPC = NBR // NCORES     # 4 branches per core
RPC = (B * SL) // NCORES  # 512 rows per core
MB = RPC // P           # 4 row blocks per core in phase 2
JC = H // P             # 24 hidden chunks
F1 = 512                # free-dim split of D=768 into 512+256

_cache: dict = {}


def _mm_r(nc, out, lhsT, rhs, start, stop):
    nc.tensor.matmul(out, lhsT=lhsT, rhs=rhs, start=start, stop=stop)


def _build_phase1(skip_conv=False, skip_proj=False, skip_norm=False):
    nc = bacc.Bacc("TRN2", target_bir_lowering=False, debug=False, num_devices=NCORES)
    x = nc.dram_tensor("x", (B, SL, D), F32, kind="ExternalInput").ap()
    tw = nc.dram_tensor("tw", (NB, P, 2, BPC * P), FP8, kind="ExternalInput").ap()
    wt = nc.dram_tensor("wt", (BPC, DC // 2, P, 2, D), FP8, kind="ExternalInput").ap()
    rn1 = nc.dram_tensor("rn1", (1, D), F32, kind="ExternalInput").ap()
    sp = nc.dram_tensor("sp", (B, SL, D), F32, kind="ExternalOutput").ap()

    with tile.TileContext(nc) as tc:
        with (
            tc.tile_pool(name="const", bufs=1) as const_pool,
            tc.tile_pool(name="ubuf", bufs=1) as ubuf_pool,
            tc.tile_pool(name="work", bufs=3) as work,
            tc.tile_pool(name="drain", bufs=2) as drain_pool,
            tc.tile_pool(name="psum_u", bufs=4, space="PSUM") as psum_u_pool,
            tc.tile_pool(name="psum_sp", bufs=2, space="PSUM") as psum_sp_pool,
        ):
            tw_sb = const_pool.tile([P, NB, 2, BPC * P], FP8)
            nc.sync.dma_start(tw_sb, tw.rearrange("d p k f -> p d k f"))
            wt_sb = const_pool.tile([P, BPC, DC // 2, 2, D], FP8)
            nc.sync.dma_start(wt_sb, wt.rearrange("b c p k f -> p b c k f"))
            rn1_bc = const_pool.tile([P, D], F32)
            nc.sync.dma_start(rn1_bc, rn1.to_broadcast((P, D)))
            eps_sb = const_pool.tile([P, 1], F32)
            nc.vector.memset(eps_sb, float(EPS))

            # u = rmsnorm(x) * rn1_w, cast to bf16, for all (b, J)
            u_all = []
            for b in range(B):
                u_all.append(ubuf_pool.tile([P, NB, D], FP8, name=f"u{b}"))
            for b in range(B):
                if skip_norm:
                    break
                for J in range(NB):
                    xt = work.tile([P, D], F32, name="xt")
                    nc.sync.dma_start(xt, x[b, J * P:(J + 1) * P, :])
                    sq = work.tile([P, D], F32, name="sq")
                    ms = work.tile([P, 1], F32, name="ms")
                    nc.scalar.activation(
                        sq, xt, mybir.ActivationFunctionType.Square, accum_out=ms
                    )
                    nc.scalar.activation(
                        ms, ms, mybir.ActivationFunctionType.Sqrt,
                        bias=eps_sb, scale=1.0 / D,
                    )
                    nc.vector.reciprocal(ms, ms)
                    nc.vector.tensor_scalar_mul(xt, xt, ms)
                    nc.vector.tensor_tensor(
                        u_all[b][:, J, :], xt, rn1_bc, mybir.AluOpType.mult
                    )

            # conv (block-Toeplitz, bf16) + projection (f32r) per (b, I)
            for b in range(B):
                for I in range(NB):
                    ut_sb = drain_pool.tile([P, DC, BPC * P], FP8, name="ut")
                    if skip_conv:
                        nc.vector.memset(ut_sb, 0.0)
                    for c in range(DC if not skip_conv else 0):
                        ps = psum_u_pool.tile([P, BPC * P], F32, name="psu")
                        npair = I // 2 + 1
                        for Jp in range(npair):
                            nc.tensor.matmul(
                                ps,
                                lhsT=u_all[b][:, 2 * Jp:2 * Jp + 2, c * P:(c + 1) * P],
                                rhs=tw_sb[:, I - 2 * Jp, :, :],
                                start=(Jp == 0),
                                stop=(Jp == npair - 1),
                                perf_mode=mybir.MatmulPerfMode.DoubleRow,
                            )
                        if c % 2 == 0:
                            nc.vector.tensor_scalar_mul(
                                ut_sb[:, c, :], ps, float(UT_SCALE / TAP_SCALE)
                            )
                        else:
                            nc.scalar.activation(
                                ut_sb[:, c, :], ps,
                                mybir.ActivationFunctionType.Copy,
                                scale=float(UT_SCALE / TAP_SCALE),
                            )
                    psp = psum_sp_pool.tile([P, D], F32, name="psp")
                    n_mm = BPC * (DC // 2)
                    i_mm = 0
                    for br in range(BPC if not skip_proj else 0):
                        for cp in range(DC // 2):
                            st = i_mm == 0
                            fin = i_mm == n_mm - 1
                            lh = ut_sb[:, 2 * cp:2 * cp + 2, br * P:(br + 1) * P]
                            nc.tensor.matmul(
                                psp[:, 0:F1], lhsT=lh,
                                rhs=wt_sb[:, br, cp, :, 0:F1],
                                start=st, stop=fin,
                                perf_mode=mybir.MatmulPerfMode.DoubleRow,
                            )
                            nc.tensor.matmul(
                                psp[:, F1:D], lhsT=lh,
                                rhs=wt_sb[:, br, cp, :, F1:D],
                                start=st, stop=fin,
                                perf_mode=mybir.MatmulPerfMode.DoubleRow,
                            )
                            i_mm += 1
                    sp_t = work.tile([P, D], F32, name="spt")
                    if skip_proj:
                        nc.vector.memset(psp, 0.0)
                    nc.vector.tensor_scalar_mul(sp_t, psp, float(1.0 / SP_SCALE))
                    nc.sync.dma_start(sp[b, I * P:(I + 1) * P, :], sp_t)
    nc.compile()
    return nc


def _build_phase2(skip_ar=False, skip_fc1=False, skip_fc2=False, skip_tr=False):
    nc = bacc.Bacc("TRN2", target_bir_lowering=False, debug=False, num_devices=NCORES)
    xr = nc.dram_tensor("xr", (RPC + 2, D), F32, kind="ExternalInput").ap()
    x1r = nc.dram_tensor("x1r", (RPC, D), F32, kind="ExternalInput").ap()
    mut = nc.dram_tensor("mut", (KU, DC, P, D), F32R, kind="ExternalInput").ap()
    fc1 = nc.dram_tensor("fc1", (D, 2 * H), F32R, kind="ExternalInput").ap()
    fc2 = nc.dram_tensor("fc2", (H, D), F32R, kind="ExternalInput").ap()
    rn1 = nc.dram_tensor("rn1", (1, D), F32, kind="ExternalInput").ap()
    rn2 = nc.dram_tensor("rn2", (1, D), F32, kind="ExternalInput").ap()
    o = nc.dram_tensor("o", (RPC, D), F32, kind="ExternalOutput").ap()

    fc1_r = fc1.rearrange("(c p) j -> p c j", p=P)
    fc2_r = fc2.rearrange("(c p) d -> p c d", p=P)

    with tile.TileContext(nc) as tc:
        with (
            tc.tile_pool(name="const", bufs=1) as const_pool,
            tc.tile_pool(name="persist", bufs=1) as persist,
            tc.tile_pool(name="work", bufs=2) as work,
            tc.tile_pool(name="wstream", bufs=2) as wstream,
            tc.tile_pool(name="psum_big", bufs=4, space="PSUM") as psum_big_pool,
            tc.tile_pool(name="w2stream", bufs=4) as w2stream,
        ):
            mut_sb = const_pool.tile([P, KU, DC, D], F32R)
            nc.sync.dma_start(mut_sb, mut.rearrange("t c p d -> p t c d"))
            rn1_bc = const_pool.tile([P, D], F32)
            nc.sync.dma_start(rn1_bc, rn1.to_broadcast((P, D)))
            rn2_bc = const_pool.tile([P, D], F32)
            nc.sync.dma_start(rn2_bc, rn2.to_broadcast((P, D)))
            ident = const_pool.tile([P, P], F32)
            make_identity(nc, ident)
            eps_sb = const_pool.tile([P, 1], F32)
            nc.vector.memset(eps_sb, float(EPS))

            u_pre = persist.tile([2, D], F32)
            ut_ext = persist.tile([P, DC, MB, P + 2], F32R)
            x1p = persist.tile([P, MB, D], F32)
            yt = persist.tile([P, DC, MB * P], F32R)
            gt = persist.tile([P, JC, MB * P], F32R)

            def rmsnorm_to(dst, src_f32, rows, w_bc):
                sq = work.tile([P, D], F32, name="sq")
                ms = work.tile([P, 1], F32, name="ms")
                nc.scalar.activation(
                    sq[:rows], src_f32[:rows],
                    mybir.ActivationFunctionType.Square, accum_out=ms[:rows],
                )
                nc.scalar.activation(
                    ms[:rows], ms[:rows], mybir.ActivationFunctionType.Sqrt,
                    bias=eps_sb[:rows], scale=1.0 / D,
                )
                nc.vector.reciprocal(ms[:rows], ms[:rows])
                tmp = sq  # sq is dead after the accumulated Square
                nc.vector.tensor_scalar_mul(tmp[:rows], src_f32[:rows], ms[:rows])
                nc.vector.tensor_tensor(
                    dst, tmp[:rows], w_bc[:rows], mybir.AluOpType.mult
                )

            # u for the 2-row prefix, then u^T per owned block via PE transpose
            xp = work.tile([P, D], F32, name="xt")[:2]
            nc.sync.dma_start(xp, xr[0:2, :])
            rmsnorm_to(u_pre, xp, 2, rn1_bc)
            for c in range(DC):
                pst2 = psum_big_pool.tile([P, D], F32, name="pbig")[:, 0:P]
                nc.tensor.transpose(
                    pst2[:, 0:2], u_pre[:, c * P:(c + 1) * P], ident[0:2, 0:2]
                )
                nc.vector.tensor_copy(ut_ext[:, c, 0, 0:2], pst2[:, 0:2])
            for m in range(MB):
                xt = work.tile([P, D], F32, name="xt")
                nc.sync.dma_start(xt, xr[2 + m * P: 2 + (m + 1) * P, :])
                uo = work.tile([P, D], F32, name="uo")
                rmsnorm_to(uo, xt, P, rn1_bc)
                for c in range(DC if not skip_tr else 0):
                    pst = psum_big_pool.tile([P, D], F32, name="pbig")[:, 0:P]
                    nc.tensor.transpose(pst, uo[:, c * P:(c + 1) * P], ident)
                    nc.vector.tensor_copy(ut_ext[:, c, m, 2:P + 2], pst)
            for m in range(1, MB):
                for c in range(DC):
                    nc.vector.tensor_copy(
                        ut_ext[:, c, m, 0:2], ut_ext[:, c, m - 1, P:P + 2]
                    )

            # AR term + x1 rows
            for m in range(MB):
                psa = psum_big_pool.tile([P, D], F32, name="pbig")
                if skip_ar:
                    nc.vector.memset(psa, 0.0)
                i_mm = 0
                n_mm = KU * DC
                for t in range(KU if not skip_ar else 0):
                    for c in range(DC):
                        st = i_mm == 0
                        fin = i_mm == n_mm - 1
                        _mm_r(nc, psa[:, 0:F1],
                              ut_ext[:, c, m, 2 - t:P + 2 - t],
                              mut_sb[:, t, c, 0:F1], st, fin)
                        _mm_r(nc, psa[:, F1:D],
                              ut_ext[:, c, m, 2 - t:P + 2 - t],
                              mut_sb[:, t, c, F1:D], st, fin)
                        i_mm += 1
                x1t = work.tile([P, D], F32, name="x1t")
                nc.sync.dma_start(x1t, x1r[m * P:(m + 1) * P, :])
                nc.vector.tensor_tensor(
                    x1p[:, m, :], x1t, psa, mybir.AluOpType.add
                )

            # y = rmsnorm2(x1) and y^T
            for m in range(MB):
                yf = work.tile([P, D], F32, name="uo")
                rmsnorm_to(yf, x1p[:, m, :], P, rn2_bc)
                for c in range(DC):
                    pst = psum_big_pool.tile([P, D], F32, name="pbig")[:, 0:P]
                    nc.tensor.transpose(pst, yf[:, c * P:(c + 1) * P], ident)
                    nc.vector.tensor_copy(yt[:, c, m * P:(m + 1) * P], pst)

            # fc1 + silu gate
            for jc in range(JC):
                fw = wstream.tile([P, DC, 2, P], F32R, name="fw")
                nc.sync.dma_start(fw[:, :, 0, :], fc1_r[:, :, jc * P:(jc + 1) * P])
                nc.sync.dma_start(
                    fw[:, :, 1, :], fc1_r[:, :, (JC + jc) * P:(JC + jc + 1) * P]
                )
                ph1 = psum_big_pool.tile([P, D], F32, name="pbig")[:, 0:F1]
                ph2 = psum_big_pool.tile([P, D], F32, name="pbig")[:, 0:F1]
                if skip_fc1:
                    nc.vector.memset(ph1, 0.0)
                    nc.vector.memset(ph2, 0.0)
                for c in range(DC if not skip_fc1 else 0):
                    _mm_r(nc, ph1, fw[:, c, 0, :], yt[:, c, :],
                          c == 0, c == DC - 1)
                    _mm_r(nc, ph2, fw[:, c, 1, :], yt[:, c, :],
                          c == 0, c == DC - 1)
                sact = work.tile([P, F1], F32, name="sact")
                nc.scalar.activation(sact, ph2, mybir.ActivationFunctionType.Silu)
                nc.vector.tensor_tensor(
                    gt[:, jc, :], ph1, sact, mybir.AluOpType.mult
                )

            # fc2 + residual: fc2 streamed exactly once, in two D-halves,
            # with one persistent PSUM accumulator per row-block
            # bank-aligned D-split (PSUM matmul regions must not cross banks)
            DSPLITS = ((0, F1), (F1, D))
            po4 = [psum_big_pool.tile([P, D], F32, name="pbig") for _ in range(MB)]
            if skip_fc2:
                for m in range(MB):
                    nc.vector.memset(po4[m], 0.0)
            for d0, d1 in (DSPLITS if not skip_fc2 else ()):
                for jc in range(JC):
                    f2w = w2stream.tile([P, F1], F32R, name="f2w")[:, :d1 - d0]
                    nc.sync.dma_start(f2w, fc2_r[:, jc, d0:d1])
                    st = jc == 0
                    fin = jc == JC - 1
                    for m in range(MB):
                        _mm_r(nc, po4[m][:, d0:d1],
                              gt[:, jc, m * P:(m + 1) * P], f2w, st, fin)
            for m in range(MB):
                ot = work.tile([P, D], F32, name="x1t")
                nc.vector.tensor_tensor(
                    ot, x1p[:, m, :], po4[m], mybir.AluOpType.add
                )
                nc.sync.dma_start(o[m * P:(m + 1) * P, :], ot)
    nc.compile()
    return nc


def _host_prep(V, sigma, M_u, M_phi_plus, M_phi_minus):
    """Per-core weight tensors: Toeplitz tap blocks + projection matrices."""
    phi = np.fft.irfft(V.astype(np.complex128), n=NFFT, axis=0)[:SL]
    s4 = sigma.astype(np.float64) ** 0.25
    alt = (-1.0) ** np.arange(SL)

    taps = np.zeros((NBR, SL))
    Wb = np.zeros((NBR, D, D), np.float32)
    for k in range(K):
        taps[2 * k] = s4[k] * phi[:, k]
        taps[2 * k + 1] = s4[k] * phi[:, k] * alt
        Wb[2 * k] = M_phi_plus[k]
        Wb[2 * k + 1] = M_phi_minus[k]

    idx = np.arange(P)
    cmr = idx[None, :] - idx[:, None]       # [r, c] = c - r
    tw_cores = []
    wt_cores = []
    for core in range(NCORES):
        brs = range(core * BPC, (core + 1) * BPC)
        # tw[d0, :, ko, :] = T-block pair (delta=d0 for ko=0, delta=d0-1 for
        # ko=1, zeros for delta<0), taps scaled by TAP_SCALE for fp8 range
        tw = np.zeros((NB, P, 2, BPC * P), np.float32)
        wt = np.zeros((BPC, DC // 2, P, 2, D), np.float32)
        for bi, br in enumerate(brs):
            tsc = taps[br] * TAP_SCALE
            for d0 in range(NB):
                for ko in range(2):
                    d = d0 - ko
                    if d < 0:
                        continue
                    ii = d * P + cmr
                    blk = np.where(ii >= 0, tsc[np.clip(ii, 0, SL - 1)], 0.0)
                    tw[d0, :, ko, bi * P:(bi + 1) * P] = blk
            for cp in range(DC // 2):
                for ko in range(2):
                    c = 2 * cp + ko
                    # wt[bi, cp, i, ko, d] = Wb[br][d, c*P + i] * W_SCALE
                    wt[bi, cp, :, ko, :] = Wb[br][:, c * P:(c + 1) * P].T * W_SCALE
        tw_cores.append(tw.astype(FP8NP))
        wt_cores.append(wt.astype(FP8NP))
    return tw_cores, wt_cores


def kernel(x, V, sigma, M_u, M_phi_plus, M_phi_minus, rn1_w, rn2_w, fc1_w, fc2_w):
    x = np.ascontiguousarray(x, np.float32)
    if "p1" not in _cache:
        _cache["p1"] = _SpmdRunner(_build_phase1(), shared=("x", "rn1"), volatile=("x",))
    if "p2" not in _cache:
        _cache["p2"] = _SpmdRunner(_build_phase2(), shared=("mut", "fc1", "fc2", "rn1", "rn2"), volatile=("xr", "x1r"))

    tw_cores, wt_cores = _host_prep(V, sigma, M_u, M_phi_plus, M_phi_minus)
    rn1 = np.ascontiguousarray(rn1_w, np.float32).reshape(1, D)
    rn2 = np.ascontiguousarray(rn2_w, np.float32).reshape(1, D)

    in_maps1 = [
        {"x": x, "tw": tw_cores[c], "wt": wt_cores[c], "rn1": rn1}
        for c in range(NCORES)
    ]
    r1 = _cache["p1"]
    sp_cat = r1.run_prepped(r1.prep(in_maps1))[0]
    if "reduce" not in _cache:
        import jax
        import jax.numpy as jnp
        from jax.sharding import NamedSharding, PartitionSpec
        mesh = r1._fn.__wrapped__ if False else None
        sh = NamedSharding(r1._shardings["x"].mesh, PartitionSpec())
        _cache["reduce"] = jax.jit(
            lambda spc, xx: xx + spc.reshape(NCORES, B, SL, D).sum(0),
            out_shardings=sh,
        )
    x1 = np.asarray(_cache["reduce"](sp_cat, r1.prep(in_maps1)[0]
                                     if False else np.asarray(x)))

    # phase 2 inputs
    mut = np.zeros((KU, DC, P, D), np.float32)
    for t in range(KU):
        for c in range(DC):
            mut[t, c] = M_u[t][:, c * P:(c + 1) * P].T
    fc1 = np.ascontiguousarray(fc1_w, np.float32)
    fc2 = np.ascontiguousarray(fc2_w, np.float32)

    x_rows = x.reshape(B * SL, D)
    x1_rows = x1.reshape(B * SL, D)
    in_maps2 = []
    for c in range(NCORES):
        r0 = c * RPC
        xr = np.zeros((RPC + 2, D), np.float32)
        xr[2:] = x_rows[r0:r0 + RPC]
        if r0 % SL != 0:
            xr[0:2] = x_rows[r0 - 2:r0]
        in_maps2.append({
            "xr": xr, "x1r": np.ascontiguousarray(x1_rows[r0:r0 + RPC]),
            "mut": mut, "fc1": fc1, "fc2": fc2, "rn1": rn1, "rn2": rn2,
        })
    res2 = _cache["p2"](in_maps2)
    out = np.concatenate(
        [res2[c]["o"] for c in range(NCORES)], axis=0
    ).reshape(B, SL, D)
    return out


# revision 30
# speedup vs baseline: 1.2370x; 1.0374x over previous
"""Trainium2 Bass kernel for the STU (spectral transform unit) dense-transformer block.

Algorithm (validated against the jax reference in fp64 numpy):
  The FFT causal conv is rewritten as a block-Toeplitz matmul. For each of the
  K=16 filters and each sign branch (the alternating-sign branch folds into the
  filter taps: T^-[s,s'] = phi[s-s'] * (-1)^(s-s')), the causal conv is
    U_br = T_br @ u,  T_br block-Toeplitz with 16 distinct 128x128 blocks.
  sigma^(1/4) folds into the taps. The (k,i)->d projection contracts U with
  M_phi_{plus,minus}; the KU=3 autoregressive taps are shifted-u projections
  with M_u. MLP is a standard gated MLP.

Sharding (8 cores, no cross-core communication, host-side reduce between two
uniform SPMD programs):
  Phase 1: filter-branch-parallel. Core c computes conv + projection for its 4
           of the 32 (k, sign) branches over the full (B, SL): partial spectral.
  Host:    x1 = x + sum_c partial_c
  Phase 2: row-parallel. Core c owns 512 of the 4096 (b, s) rows: adds the AR
           term and computes the gated MLP + residual for its rows.

Precision: the conv runs in bf16 (its output feeds values of magnitude ~0.05,
so bf16 noise is negligible); every O(1)-magnitude contraction (projection,
AR, fc1, fc2) runs in float32r — fp32 storage at full PE rate for moving
dims >= 256, measured ~15x more accurate than bf16.
"""

import numpy as np
import ml_dtypes

import concourse.bacc as bacc
import concourse.tile as tile
from concourse import mybir
from concourse.bass_utils import run_bass_kernel_spmd  # noqa: F401 (debug path)
from concourse.masks import make_identity


class _SpmdRunner:
    """Cached-jit SPMD executor: trace/compile once, then repeat calls only
    pay input upload + execution (mirrors bass2jax.run_bass_via_pjrt).

    ``shared`` names inputs that are identical on every core: they are fed
    replicated (host uploads one copy) instead of 8x-concatenated."""

    def __init__(self, nc, shared=(), volatile=()):
        import jax
        import concourse.mybir as _mb
        from concourse.bass2jax import (
            install_neuronx_cc_hook, _bass_exec_p, partition_id_tensor,
        )
        from jax.experimental.shard_map import shard_map
        from jax.sharding import Mesh, PartitionSpec

        install_neuronx_cc_hook()
        self.nc = nc
        assert nc.dbg_addr is None
        pid_name = (nc.partition_id_tensor.name
                    if nc.partition_id_tensor is not None else None)
        in_names, out_names, out_avals = [], [], []
        for alloc in nc.m.functions[0].allocations:
            if not isinstance(alloc, mybir.MemoryLocationSet):
                continue
            name = alloc.memorylocations[0].name
            if alloc.kind == "ExternalInput":
                if name != pid_name:
                    in_names.append(name)
            elif alloc.kind == "ExternalOutput":
                out_names.append(name)
                out_avals.append(jax.core.ShapedArray(
                    tuple(alloc.tensor_shape), mybir.dt.np(alloc.dtype)))
        self.in_names, self.out_names, self.out_avals = in_names, out_names, out_avals
        self.shared = frozenset(shared)
        self.volatile = frozenset(volatile)
        self._dev_cache = {}
        n_params = len(in_names)
        all_names = tuple(in_names + out_names)
        if pid_name is not None:
            all_names = all_names + (pid_name,)

        def _body(*args):
            args = list(args)
            if pid_name is not None:
                args.append(partition_id_tensor())
            return tuple(_bass_exec_p.bind(
                *args,
                out_avals=tuple(out_avals),
                in_names=all_names,
                out_names=tuple(out_names),
                lowering_input_output_aliases=(),
                sim_require_finite=True,
                sim_require_nnan=True,
                nc=nc,
            ))

        import jax.numpy as jnp
        from jax.sharding import NamedSharding
        devices = jax.devices()[:NCORES]
        mesh = Mesh(np.asarray(devices), ("core",))
        rep = PartitionSpec()
        core = PartitionSpec("core")
        in_specs = tuple(
            rep if nm in self.shared else core for nm in in_names
        ) + (core,) * len(out_names)
        out_specs = (core,) * len(out_names)
        donate = tuple(range(n_params, n_params + len(out_names)))
        self._fn = jax.jit(
            shard_map(_body, mesh=mesh, in_specs=in_specs, out_specs=out_specs,
                      check_rep=False),
            donate_argnums=donate, keep_unused=True,
        )
        self._zeros_fn = jax.jit(
            lambda: tuple(
                jnp.zeros((NCORES * a.shape[0], *a.shape[1:]), a.dtype)
                for a in out_avals
            ),
            out_shardings=tuple(
                NamedSharding(mesh, core) for _ in out_avals
            ),
        )
        self._shardings = {
            nm: NamedSharding(mesh, rep if nm in self.shared else core)
            for nm in in_names
        }

    def prep(self, in_maps):
        import hashlib
        import jax
        ins = []
        for nm in self.in_names:
            if nm in self.shared:
                arr = np.ascontiguousarray(in_maps[0][nm])
            else:
                arr = np.concatenate(
                    [np.asarray(in_maps[c][nm]) for c in range(NCORES)], axis=0)
            if nm in self.volatile:
                ins.append(arr)
                continue
            key = (nm, hashlib.md5(arr.tobytes()).hexdigest())
            dev = self._dev_cache.get(key)
            if dev is None:
                self._dev_cache.clear() if len(self._dev_cache) > 32 else None
                dev = jax.device_put(arr, self._shardings[nm])
                self._dev_cache[key] = dev
            ins.append(dev)
        return ins

    def run_prepped(self, ins):
        return self._fn(*ins, *self._zeros_fn())

    def __call__(self, in_maps):
        out_arrs = self.run_prepped(self.prep(in_maps))
        return [
            {nm: np.asarray(out_arrs[i]).reshape(NCORES, *self.out_avals[i].shape)[c]
             for i, nm in enumerate(self.out_names)}
            for c in range(NCORES)
        ]

BF16 = ml_dtypes.bfloat16
FP8NP = ml_dtypes.float8_e4m3
TAP_SCALE = 1024.0
UT_SCALE = 32.0      # psum (TAP_SCALE*U) -> fp8 ut tiles scale factor: 32/1024
W_SCALE = 16.0       # projection weights scaled by 16 for fp8 range
SP_SCALE = UT_SCALE * W_SCALE  # spectral psum carries 32*16 = 512x
F32 = mybir.dt.float32
F32R = mybir.dt.float32r
BF = mybir.dt.bfloat16
FP8 = mybir.dt.float8e4

B, SL, D, K, KU = 2, 2048, 768, 16, 3
NFFT, EPS, P, H = 4096, 1e-5, 128, 3072
NB = SL // P            # 16 seq blocks
DC = D // P             # 6 d-chunks
NBR = 2 * K             # 32 conv branches
NCORES = 8
BPC = NBR // NCORES     # 4 branches per core
RPC = (B * SL) // NCORES  # 512 rows per core
MB = RPC // P           # 4 row blocks per core in phase 2
JC = H // P             # 24 hidden chunks
F1 = 512                # free-dim split of D=768 into 512+256

_cache: dict = {}


def _mm_r(nc, out, lhsT, rhs, start, stop):
    nc.tensor.matmul(out, lhsT=lhsT, rhs=rhs, start=start, stop=stop)


def _build_phase1(skip_conv=False, skip_proj=False, skip_norm=False):
    nc = bacc.Bacc("TRN2", target_bir_lowering=False, debug=False, num_devices=NCORES)
    x = nc.dram_tensor("x", (B, SL, D), F32, kind="ExternalInput").ap()
    tw = nc.dram_tensor("tw", (NB, P, 2, BPC * P), FP8, kind="ExternalInput").ap()
    wt = nc.dram_tensor("wt", (BPC, DC // 2, P, 2, D), FP8, kind="ExternalInput").ap()
    rn1 = nc.dram_tensor("rn1", (1, D), F32, kind="ExternalInput").ap()
    sp = nc.dram_tensor("sp", (B, SL, D), F32, kind="ExternalOutput").ap()

    with tile.TileContext(nc) as tc:
        with (
            tc.tile_pool(name="const", bufs=1) as const_pool,
            tc.tile_pool(name="ubuf", bufs=1) as ubuf_pool,
            tc.tile_pool(name="work", bufs=3) as work,
            tc.tile_pool(name="drain", bufs=3) as drain_pool,
            tc.tile_pool(name="psum_u", bufs=4, space="PSUM") as psum_u_pool,
            tc.tile_pool(name="psum_sp", bufs=2, space="PSUM") as psum_sp_pool,
        ):
            tw_sb = const_pool.tile([P, NB, 2, BPC * P], FP8)
            nc.sync.dma_start(tw_sb, tw.rearrange("d p k f -> p d k f"))
            wt_sb = const_pool.tile([P, BPC, DC // 2, 2, D], FP8)
            nc.sync.dma_start(wt_sb, wt.rearrange("b c p k f -> p b c k f"))
            rn1_bc = const_pool.tile([P, D], F32)
            nc.sync.dma_start(rn1_bc, rn1.to_broadcast((P, D)))
            eps_sb = const_pool.tile([P, 1], F32)
            nc.vector.memset(eps_sb, float(EPS))

            # u = rmsnorm(x) * rn1_w, cast to bf16, for all (b, J)
            u_all = []
            for b in range(B):
                u_all.append(ubuf_pool.tile([P, NB, D], FP8, name=f"u{b}"))
            for b in range(B):
                if skip_norm:
                    break
                for J in range(NB):
                    xt = work.tile([P, D], F32, name="xt")
                    nc.sync.dma_start(xt, x[b, J * P:(J + 1) * P, :])
                    sq = work.tile([P, D], F32, name="sq")
                    ms = work.tile([P, 1], F32, name="ms")
                    nc.scalar.activation(
                        sq, xt, mybir.ActivationFunctionType.Square, accum_out=ms
                    )
                    nc.scalar.activation(
                        ms, ms, mybir.ActivationFunctionType.Sqrt,
                        bias=eps_sb, scale=1.0 / D,
                    )
                    nc.vector.reciprocal(ms, ms)
                    nc.vector.tensor_scalar_mul(xt, xt, ms)
                    nc.vector.tensor_tensor(
                        u_all[b][:, J, :], xt, rn1_bc, mybir.AluOpType.mult
                    )

            # conv (block-Toeplitz, bf16) + projection (f32r) per (b, I)
            for b in range(B):
                for I in range(NB):
                    ut_sb = drain_pool.tile([P, DC, BPC * P], FP8, name="ut")
                    if skip_conv:
                        nc.vector.memset(ut_sb, 0.0)
                    for c in range(DC if not skip_conv else 0):
                        ps = psum_u_pool.tile([P, BPC * P], F32, name="psu")
                        npair = I // 2 + 1
                        for Jp in range(npair):
                            nc.tensor.matmul(
                                ps,
                                lhsT=u_all[b][:, 2 * Jp:2 * Jp + 2, c * P:(c + 1) * P],
                                rhs=tw_sb[:, I - 2 * Jp, :, :],
                                start=(Jp == 0),
                                stop=(Jp == npair - 1),
                                perf_mode=mybir.MatmulPerfMode.DoubleRow,
                            )
                        if c % 2 == 0:
                            nc.vector.tensor_scalar_mul(
                                ut_sb[:, c, :], ps, float(UT_SCALE / TAP_SCALE)
                            )
                        else:
                            nc.scalar.activation(
                                ut_sb[:, c, :], ps,
                                mybir.ActivationFunctionType.Copy,
                                scale=float(UT_SCALE / TAP_SCALE),
                            )
                    psp = psum_sp_pool.tile([P, D], F32, name="psp")
                    n_mm = BPC * (DC // 2)
                    i_mm = 0
                    for br in range(BPC if not skip_proj else 0):
                        for cp in range(DC // 2):
                            st = i_mm == 0
                            fin = i_mm == n_mm - 1
                            lh = ut_sb[:, 2 * cp:2 * cp + 2, br * P:(br + 1) * P]
                            nc.tensor.matmul(
                                psp[:, 0:F1], lhsT=lh,
                                rhs=wt_sb[:, br, cp, :, 0:F1],
                                start=st, stop=fin,
                                perf_mode=mybir.MatmulPerfMode.DoubleRow,
                            )
                            nc.tensor.matmul(
                                psp[:, F1:D], lhsT=lh,
                                rhs=wt_sb[:, br, cp, :, F1:D],
                                start=st, stop=fin,
                                perf_mode=mybir.MatmulPerfMode.DoubleRow,
                            )
                            i_mm += 1
                    sp_t = work.tile([P, D], F32, name="spt")
                    if skip_proj:
                        nc.vector.memset(psp, 0.0)
                    nc.vector.tensor_scalar_mul(sp_t, psp, float(1.0 / SP_SCALE))
                    nc.sync.dma_start(sp[b, I * P:(I + 1) * P, :], sp_t)
    nc.compile()
    return nc


def _build_phase2(skip_ar=False, skip_fc1=False, skip_fc2=False, skip_tr=False):
    nc = bacc.Bacc("TRN2", target_bir_lowering=False, debug=False, num_devices=NCORES)
    xr = nc.dram_tensor("xr", (RPC + 2, D), F32, kind="ExternalInput").ap()
    x1r = nc.dram_tensor("x1r", (RPC, D), F32, kind="ExternalInput").ap()
    mut = nc.dram_tensor("mut", (KU, DC, P, D), F32R, kind="ExternalInput").ap()
    fc1 = nc.dram_tensor("fc1", (D, 2 * H), F32R, kind="ExternalInput").ap()
    fc2 = nc.dram_tensor("fc2", (H, D), F32R, kind="ExternalInput").ap()
    rn1 = nc.dram_tensor("rn1", (1, D), F32, kind="ExternalInput").ap()
    rn2 = nc.dram_tensor("rn2", (1, D), F32, kind="ExternalInput").ap()
    o = nc.dram_tensor("o", (RPC, D), F32, kind="ExternalOutput").ap()

    fc1_r = fc1.rearrange("(c p) j -> p c j", p=P)
    fc2_r = fc2.rearrange("(c p) d -> p c d", p=P)

    with tile.TileContext(nc) as tc:
        with (
            tc.tile_pool(name="const", bufs=1) as const_pool,
            tc.tile_pool(name="persist", bufs=1) as persist,
            tc.tile_pool(name="work", bufs=2) as work,
            tc.tile_pool(name="wstream", bufs=3) as wstream,
            tc.tile_pool(name="psum_big", bufs=4, space="PSUM") as psum_big_pool,
            tc.tile_pool(name="w2stream", bufs=6) as w2stream,
        ):
            mut_sb = const_pool.tile([P, KU, DC, D], F32R)
            nc.sync.dma_start(mut_sb, mut.rearrange("t c p d -> p t c d"))
            rn1_bc = const_pool.tile([P, D], F32)
            nc.sync.dma_start(rn1_bc, rn1.to_broadcast((P, D)))
            rn2_bc = const_pool.tile([P, D], F32)
            nc.sync.dma_start(rn2_bc, rn2.to_broadcast((P, D)))
            ident = const_pool.tile([P, P], F32)
            make_identity(nc, ident)
            eps_sb = const_pool.tile([P, 1], F32)
            nc.vector.memset(eps_sb, float(EPS))

            u_pre = persist.tile([2, D], F32)
            ut_ext = persist.tile([P, DC, MB, P + 2], F32R)
            x1p = persist.tile([P, MB, D], F32)
            yt = persist.tile([P, DC, MB * P], F32R)
            gt = persist.tile([P, JC, MB * P], F32R)

            def rmsnorm_to(dst, src_f32, rows, w_bc):
                sq = work.tile([P, D], F32, name="sq")
                ms = work.tile([P, 1], F32, name="ms")
                nc.scalar.activation(
                    sq[:rows], src_f32[:rows],
                    mybir.ActivationFunctionType.Square, accum_out=ms[:rows],
                )
                nc.scalar.activation(
                    ms[:rows], ms[:rows], mybir.ActivationFunctionType.Sqrt,
                    bias=eps_sb[:rows], scale=1.0 / D,
                )
                nc.vector.reciprocal(ms[:rows], ms[:rows])
                tmp = sq  # sq is dead after the accumulated Square
                nc.vector.tensor_scalar_mul(tmp[:rows], src_f32[:rows], ms[:rows])
                nc.vector.tensor_tensor(
                    dst, tmp[:rows], w_bc[:rows], mybir.AluOpType.mult
                )

            # u for the 2-row prefix, then u^T per owned block via PE transpose
            xp = work.tile([P, D], F32, name="xt")[:2]
            nc.sync.dma_start(xp, xr[0:2, :])
            rmsnorm_to(u_pre, xp, 2, rn1_bc)
            for c in range(DC):
                pst2 = psum_big_pool.tile([P, D], F32, name="pbig")[:, 0:P]
                nc.tensor.transpose(
                    pst2[:, 0:2], u_pre[:, c * P:(c + 1) * P], ident[0:2, 0:2]
                )
                nc.vector.tensor_copy(ut_ext[:, c, 0, 0:2], pst2[:, 0:2])
            for m in range(MB):
                xt = work.tile([P, D], F32, name="xt")
                nc.sync.dma_start(xt, xr[2 + m * P: 2 + (m + 1) * P, :])
                uo = work.tile([P, D], F32, name="uo")
                rmsnorm_to(uo, xt, P, rn1_bc)
                for c in range(DC if not skip_tr else 0):
                    pst = psum_big_pool.tile([P, D], F32, name="pbig")[:, 0:P]
                    nc.tensor.transpose(pst, uo[:, c * P:(c + 1) * P], ident)
                    nc.vector.tensor_copy(ut_ext[:, c, m, 2:P + 2], pst)
            for m in range(1, MB):
                for c in range(DC):
                    nc.vector.tensor_copy(
                        ut_ext[:, c, m, 0:2], ut_ext[:, c, m - 1, P:P + 2]
                    )

            # AR term + x1 rows
            for m in range(MB):
                psa = psum_big_pool.tile([P, D], F32, name="pbig")
                if skip_ar:
                    nc.vector.memset(psa, 0.0)
                i_mm = 0
                n_mm = KU * DC
                for t in range(KU if not skip_ar else 0):
                    for c in range(DC):
                        st = i_mm == 0
                        fin = i_mm == n_mm - 1
                        _mm_r(nc, psa[:, 0:F1],
                              ut_ext[:, c, m, 2 - t:P + 2 - t],
                              mut_sb[:, t, c, 0:F1], st, fin)
                        _mm_r(nc, psa[:, F1:D],
                              ut_ext[:, c, m, 2 - t:P + 2 - t],
                              mut_sb[:, t, c, F1:D], st, fin)
                        i_mm += 1
                x1t = work.tile([P, D], F32, name="x1t")
                nc.sync.dma_start(x1t, x1r[m * P:(m + 1) * P, :])
                nc.vector.tensor_tensor(
                    x1p[:, m, :], x1t, psa, mybir.AluOpType.add
                )

            # y = rmsnorm2(x1) and y^T
            for m in range(MB):
                yf = work.tile([P, D], F32, name="uo")
                rmsnorm_to(yf, x1p[:, m, :], P, rn2_bc)
                for c in range(DC):
                    pst = psum_big_pool.tile([P, D], F32, name="pbig")[:, 0:P]
                    nc.tensor.transpose(pst, yf[:, c * P:(c + 1) * P], ident)
                    nc.vector.tensor_copy(yt[:, c, m * P:(m + 1) * P], pst)

            # fc1 + silu gate
            for jc in range(JC):
                fw = wstream.tile([P, DC, 2, P], F32R, name="fw")
                nc.sync.dma_start(fw[:, :, 0, :], fc1_r[:, :, jc * P:(jc + 1) * P])
                nc.sync.dma_start(
                    fw[:, :, 1, :], fc1_r[:, :, (JC + jc) * P:(JC + jc + 1) * P]
                )
                ph1 = psum_big_pool.tile([P, D], F32, name="pbig")[:, 0:F1]
                ph2 = psum_big_pool.tile([P, D], F32, name="pbig")[:, 0:F1]
                if skip_fc1:
                    nc.vector.memset(ph1, 0.0)
                    nc.vector.memset(ph2, 0.0)
                for c in range(DC if not skip_fc1 else 0):
                    _mm_r(nc, ph1, fw[:, c, 0, :], yt[:, c, :],
                          c == 0, c == DC - 1)
                    _mm_r(nc, ph2, fw[:, c, 1, :], yt[:, c, :],
                          c == 0, c == DC - 1)
                sact = work.tile([P, F1], F32, name="sact")
                nc.scalar.activation(sact, ph2, mybir.ActivationFunctionType.Silu)
                nc.vector.tensor_tensor(
                    gt[:, jc, :], ph1, sact, mybir.AluOpType.mult
                )

            # fc2 + residual: fc2 streamed exactly once, in two D-halves,
            # with one persistent PSUM accumulator per row-block
            # bank-aligned D-split (PSUM matmul regions must not cross banks)
            DSPLITS = ((0, F1), (F1, D))
            po4 = [psum_big_pool.tile([P, D], F32, name="pbig") for _ in range(MB)]
            if skip_fc2:
                for m in range(MB):
                    nc.vector.memset(po4[m], 0.0)
            for d0, d1 in (DSPLITS if not skip_fc2 else ()):
                for jc in range(JC):
                    f2w = w2stream.tile([P, F1], F32R, name="f2w")[:, :d1 - d0]
                    nc.sync.dma_start(f2w, fc2_r[:, jc, d0:d1])
                    st = jc == 0
                    fin = jc == JC - 1
                    for m in range(MB):
                        _mm_r(nc, po4[m][:, d0:d1],
                              gt[:, jc, m * P:(m + 1) * P], f2w, st, fin)
            for m in range(MB):
                ot = work.tile([P, D], F32, name="x1t")
                nc.vector.tensor_tensor(
                    ot, x1p[:, m, :], po4[m], mybir.AluOpType.add
                )
                nc.sync.dma_start(o[m * P:(m + 1) * P, :], ot)
    nc.compile()
    return nc


def _host_prep(V, sigma, M_u, M_phi_plus, M_phi_minus):
    """Per-core weight tensors: Toeplitz tap blocks + projection matrices."""
    phi = np.fft.irfft(V.astype(np.complex128), n=NFFT, axis=0)[:SL]
    s4 = sigma.astype(np.float64) ** 0.25
    alt = (-1.0) ** np.arange(SL)

    taps = np.zeros((NBR, SL))
    Wb = np.zeros((NBR, D, D), np.float32)
    for k in range(K):
        taps[2 * k] = s4[k] * phi[:, k]
        taps[2 * k + 1] = s4[k] * phi[:, k] * alt
        Wb[2 * k] = M_phi_plus[k]
        Wb[2 * k + 1] = M_phi_minus[k]

    idx = np.arange(P)
    cmr = idx[None, :] - idx[:, None]       # [r, c] = c - r
    tw_cores = []
    wt_cores = []
    for core in range(NCORES):
        brs = range(core * BPC, (core + 1) * BPC)
        # tw[d0, :, ko, :] = T-block pair (delta=d0 for ko=0, delta=d0-1 for
        # ko=1, zeros for delta<0), taps scaled by TAP_SCALE for fp8 range
        tw = np.zeros((NB, P, 2, BPC * P), np.float32)
        wt = np.zeros((BPC, DC // 2, P, 2, D), np.float32)
        for bi, br in enumerate(brs):
            tsc = taps[br] * TAP_SCALE
            for d0 in range(NB):
                for ko in range(2):
                    d = d0 - ko
                    if d < 0:
                        continue
                    ii = d * P + cmr
                    blk = np.where(ii >= 0, tsc[np.clip(ii, 0, SL - 1)], 0.0)
                    tw[d0, :, ko, bi * P:(bi + 1) * P] = blk
            for cp in range(DC // 2):
                for ko in range(2):
                    c = 2 * cp + ko
                    # wt[bi, cp, i, ko, d] = Wb[br][d, c*P + i] * W_SCALE
                    wt[bi, cp, :, ko, :] = Wb[br][:, c * P:(c + 1) * P].T * W_SCALE
        tw_cores.append(tw.astype(FP8NP))
        wt_cores.append(wt.astype(FP8NP))
    return tw_cores, wt_cores


def kernel(x, V, sigma, M_u, M_phi_plus, M_phi_minus, rn1_w, rn2_w, fc1_w, fc2_w):
    x = np.ascontiguousarray(x, np.float32)
    if "p1" not in _cache:
        _cache["p1"] = _SpmdRunner(_build_phase1(), shared=("x", "rn1"), volatile=("x",))
    if "p2" not in _cache:
        _cache["p2"] = _SpmdRunner(_build_phase2(), shared=("mut", "fc1", "fc2", "rn1", "rn2"), volatile=("xr", "x1r"))

    tw_cores, wt_cores = _host_prep(V, sigma, M_u, M_phi_plus, M_phi_minus)
    rn1 = np.ascontiguousarray(rn1_w, np.float32).reshape(1, D)
    rn2 = np.ascontiguousarray(rn2_w, np.float32).reshape(1, D)

    in_maps1 = [
        {"x": x, "tw": tw_cores[c], "wt": wt_cores[c], "rn1": rn1}
        for c in range(NCORES)
    ]
    r1 = _cache["p1"]
    sp_cat = r1.run_prepped(r1.prep(in_maps1))[0]
    if "reduce" not in _cache:
        import jax
        import jax.numpy as jnp
        from jax.sharding import NamedSharding, PartitionSpec
        mesh = r1._fn.__wrapped__ if False else None
        sh = NamedSharding(r1._shardings["x"].mesh, PartitionSpec())
        _cache["reduce"] = jax.jit(
            lambda spc, xx: xx + spc.reshape(NCORES, B, SL, D).sum(0),
            out_shardings=sh,
        )
    x1 = np.asarray(_cache["reduce"](sp_cat, r1.prep(in_maps1)[0]
                                     if False else np.asarray(x)))

    # phase 2 inputs
    mut = np.zeros((KU, DC, P, D), np.float32)
    for t in range(KU):
        for c in range(DC):
            mut[t, c] = M_u[t][:, c * P:(c + 1) * P].T
    fc1 = np.ascontiguousarray(fc1_w, np.float32)
    fc2 = np.ascontiguousarray(fc2_w, np.float32)

    x_rows = x.reshape(B * SL, D)
    x1_rows = x1.reshape(B * SL, D)
    in_maps2 = []
    for c in range(NCORES):
        r0 = c * RPC
        xr = np.zeros((RPC + 2, D), np.float32)
        xr[2:] = x_rows[r0:r0 + RPC]
        if r0 % SL != 0:
            xr[0:2] = x_rows[r0 - 2:r0]
        in_maps2.append({
            "xr": xr, "x1r": np.ascontiguousarray(x1_rows[r0:r0 + RPC]),
            "mut": mut, "fc1": fc1, "fc2": fc2, "rn1": rn1, "rn2": rn2,
        })
    res2 = _cache["p2"](in_maps2)
    out = np.concatenate(
        [res2[c]["o"] for c in range(NCORES)], axis=0
    ).reshape(B, SL, D)
    return out
